# revision 27
# baseline (speedup 1.0000x reference)
"""Multi-head attention layer (B=4, S=2048, D=1024, H=16, DH=64) on 8 TRN2 cores.

Sharding: core c = (batch b, head-group g) with b = c//2, g = c%2.
Each core computes QKV projections for one batch with an 8-head column slice
of the weights, then full attention for those 8 heads - zero collectives.

Per-core design (v2):
  - x is passed host-transposed as xT (D, S) so the D-contraction sits on
    SBUF partitions for all three projection matmuls without device
    transposes.
  - key/value sequence compaction: masked key positions contribute exactly 0
    to the softmax (exp(-10000 + s) == 0 in f32), so the host gathers the
    unmasked positions (padded with masked ones to a static NKV) into a
    separate compacted operand xkT used for the K/V projections. NKV is the
    active-key count rounded up to 128 (no 512-padding: projection pieces of
    {512,384,256} keep the f32r/bf16 moving dim >= 256).
  - all projection inputs (xT, xkT, wq, wk, wv) are bf16: same PE rate as
    f32r but half the DMA footprint, so the front-loaded weight/activation
    streams land sooner and the first exp starts earlier. Accumulation stays
    fp32 in PSUM; the softmax mean error from bf16 inputs is ~3e-3 rel.
  - q, k are produced transposed (qT/kT bf16: head_dim on partitions, seq
    free) so the scores matmul contracts over DH=64 directly; two heads
    share the PE array concurrently via row tile_position packing (base
    partitions 0 / 64).
  - scores are computed transposed (kv on partitions, q free); the mask
    adder is a per-partition bias and the softmax exp is a single ScalarE
    activation (exp(0.125*s + adder)) straight out of PSUM.
  - v is produced in natural layout (kv position on partitions) into a wide
    per-head block [v(64 cols) | ones(64 cols)]: the context matmul runs at
    M=128 (same N-cycles as M=65) and rows 64:127 of the ctx PSUM tile come
    out holding the softmax denominator replicated 64x - i.e. already
    broadcast. Normalization is then pure elementwise DVE work
    (reciprocal_approx_fast + multiply); no PE broadcast matmuls.
  - emission order: K-proj, q chunk 0, then attention for q chunk 0 with the
    V-projection chunks interleaved into the first head-pair's m-loop. This
    starts the ScalarE exp stream (the co-critical path, ~123us of work) at
    ~22us instead of ~46us.
  - output is written as ctxT (HD, S) f32; host transposes.

`reps` repeats the whole compute body inside one NEFF (used only for
device-time measurement: wall(reps=k) - wall(reps=1) isolates body time
from host/RPC overhead).
"""

import sys

import numpy as np

sys.path.insert(0, "/opt/trn_rl_repo")

B, S, D = 4, 2048, 1024
H, DH = 16, 64
HPC = 8            # heads per core
HD = HPC * DH      # 512: output columns per core
NCORES = 8
KD = D // 128      # 8 contraction chunks
NT = HD // 128     # 4 head-dim partition chunks (= head pairs)
NC4 = S // 512     # 4 q chunks of 512
VW = 2 * DH        # 128: v columns per head incl. the ones block

_CACHED = {}


def _pieces(n):
    """Split n into pieces of 512 with any tail split so every piece is a
    multiple of 128 and >= 256 (bf16/f32r matmuls want a moving dim >= 256
    for full rate)."""
    out, off = [], 0
    while n - off > 768:
        out.append((off, 512))
        off += 512
    rem = n - off
    if rem > 512:
        out.append((off, rem - 256))
        out.append((off + rem - 256, 256))
    elif rem:
        out.append((off, rem))
    assert all(w >= 256 and w % 128 == 0 for _, w in out), out
    return out


def _build_nc(nkv, nmk_attn=None, reps=1, has_bv=True, taps=False):
    from concourse import bacc, mybir, tile

    f32 = mybir.dt.float32
    f32r = mybir.dt.float32r
    bf16 = mybir.dt.bfloat16
    i32 = mybir.dt.int32
    EXP = mybir.ActivationFunctionType.Exp
    MULT = mybir.AluOpType.mult
    ADD = mybir.AluOpType.add

    NMK = nkv // 128          # kv chunks of 128 (projection width)
    if nmk_attn is None:
        nmk_attn = NMK        # attention window in 128-chunks
    assert nmk_attn <= NMK
    kv_pieces = _pieces(nkv)

    nc = bacc.Bacc("TRN2", target_bir_lowering=False, debug=False,
                   enable_asserts=False)

    xt_d = nc.declare_dram_parameter("xt", [D, S], bf16, isOutput=False)
    xkt_d = nc.declare_dram_parameter("xkt", [D, nkv], bf16, isOutput=False)
    wq_d = nc.declare_dram_parameter("wq", [D, HD], bf16, isOutput=False)
    wk_d = nc.declare_dram_parameter("wk", [D, HD], bf16, isOutput=False)
    wv_d = nc.declare_dram_parameter("wv", [D, HD], bf16, isOutput=False)
    bq_d = nc.declare_dram_parameter("bq", [HD], f32, isOutput=False)
    bk_d = nc.declare_dram_parameter("bk", [HD], f32, isOutput=False)
    bv_d = (nc.declare_dram_parameter("bv", [HD], f32r, isOutput=False)
            if has_bv else None)
    mask_d = nc.declare_dram_parameter("maskc", [nkv], i32, isOutput=False)
    out_d = nc.declare_dram_parameter("out", [HD, S], f32, isOutput=True)
    if taps:
        dva_d = nc.declare_dram_parameter("dva", [128, NMK * HPC * VW], f32,
                                          isOutput=True)

    with tile.TileContext(nc) as tc:
        with (
            tc.tile_pool(name="const", bufs=1) as cpool,
            tc.tile_pool(name="qk", bufs=1) as qkpool,
            tc.tile_pool(name="vv", bufs=1) as vpool,
            tc.tile_pool(name="outp", bufs=3) as opool,
            tc.tile_pool(name="bcp", bufs=2) as bcpool,
            tc.tile_pool(name="rcp", bufs=2) as rcpool,
        ):
            ones_f = cpool.tile([128, 128], f32)
            nc.vector.memset(ones_f[:], 1.0)
            ones_r = cpool.tile([1, 128], f32r)
            nc.vector.tensor_copy(ones_r[:], ones_f[0:1, :])
            ones_w = cpool.tile([128, HD], f32)
            nc.vector.memset(ones_w[:], 1.0)
            # compacted mask (nkv,) int32 -> additive bias tile (128, NMK):
            # adder[p, m] = (maskc[m*128+p] - 1) * 10000
            mask_t = cpool.tile([128, NMK], i32)
            nc.sync.dma_start(
                mask_t[:], mask_d.ap().rearrange("(m p) -> p m", p=128))
            maskf = cpool.tile([128, NMK], f32)
            nc.vector.tensor_copy(maskf[:], mask_t[:])
            adder = cpool.tile([128, NMK], f32)
            nc.vector.tensor_scalar(adder[:], maskf[:], 10000.0, -10000.0,
                                    MULT, ADD)

            # biases: bq/bk as per-partition columns, bv as a 1-row vector
            bq_t = cpool.tile([128, NT], f32)
            nc.sync.dma_start(
                bq_t[:], bq_d.ap().rearrange("(t p) -> p t", p=128))
            bk_t = cpool.tile([128, NT], f32)
            nc.sync.dma_start(
                bk_t[:], bk_d.ap().rearrange("(t p) -> p t", p=128))
            if has_bv:
                bv_r = cpool.tile([1, HD], f32r)
                nc.sync.dma_start(bv_r[:],
                                  bv_d.ap().rearrange("(o n) -> o n", o=1))

            qT = qkpool.tile([128, NT * S], bf16)    # q transposed
            kT = qkpool.tile([128, NT * nkv], bf16)  # k transposed, compacted
            # v' blocks per (m, head): [ones cols 0:64 | v cols 64:128] so the
            # ctx PSUM tile holds the denominator on partitions 0:63 (the
            # custom DVE reciprocal can only read from base partition 0).
            vA = vpool.tile([128, NMK * HPC * VW], f32r)
            for m in range(NMK):
                nc.vector.tensor_copy(
                    vA[:, m * HPC * VW:(m + 1) * HPC * VW]
                    .rearrange("p (h e) -> p h e", h=HPC)[:, :, 0:DH],
                    ones_w[:].rearrange("p (h e) -> p h e", h=HPC))

            for rep in range(reps):
                with (
                    tc.tile_pool(name=f"xtp{rep}", bufs=12) as xpool,
                    tc.tile_pool(name=f"xkp{rep}", bufs=26) as xkpool,
                    tc.tile_pool(name=f"wp{rep}", bufs=1) as wpool,
                    tc.tile_pool(name=f"probs{rep}", bufs=8) as ppool,
                    tc.tile_pool(name=f"psq{rep}", bufs=2,
                                 space="PSUM") as psq,
                    tc.tile_pool(name=f"pssc{rep}", bufs=2,
                                 space="PSUM") as pssc,
                    tc.tile_pool(name=f"psctx{rep}", bufs=2,
                                 space="PSUM") as psctx,
                ):
                    wqt = wpool.tile([128, KD * HD], bf16)
                    wkt = wpool.tile([128, KD * HD], bf16)
                    wvt = wpool.tile([128, KD * HD], bf16)

                    def dma_w(wt, wd):
                        for d in range(KD):
                            nc.sync.dma_start(
                                wt[:, d * HD:(d + 1) * HD],
                                wd.ap()[d * 128:(d + 1) * 128, :])

                    # ---------------- K projection ----------------
                    # wk chunk DMAs interleave with the first xk piece so
                    # round-robin queue assignment transfers them in parallel.
                    xk_pieces = []
                    for pi, (off, w) in enumerate(kv_pieces):
                        xkp = []
                        for d in range(KD):
                            if pi == 0:
                                nc.sync.dma_start(
                                    wkt[:, d * HD:(d + 1) * HD],
                                    wk_d.ap()[d * 128:(d + 1) * 128, :])
                            t_ = xkpool.tile([128, 512], bf16, tag="xk")
                            nc.sync.dma_start(
                                t_[:, 0:w],
                                xkt_d.ap()[d * 128:(d + 1) * 128,
                                           off:off + w])
                            xkp.append(t_)
                        xk_pieces.append(xkp)
                        if pi == 0:
                            dma_w(wqt, wq_d)
                        for t in range(NT):
                            ps = psq.tile([128, 512], f32, tag="psqkv")
                            for d in range(KD):
                                nc.tensor.matmul(
                                    ps[:, 0:w],
                                    wkt[:, d * HD + t * 128:
                                        d * HD + (t + 1) * 128],
                                    xkp[d][:, 0:w],
                                    start=(d == 0), stop=(d == KD - 1))
                            nc.vector.tensor_scalar_add(
                                kT[:, t * nkv + off:t * nkv + off + w],
                                ps[:, 0:w], bk_t[:, t:t + 1])

                    qstate = {"xp": None, "c4": -1}

                    def qproj_t(c4, t):
                        if qstate["c4"] != c4:
                            xp = []
                            for d in range(KD):
                                t_ = xpool.tile([128, 512], bf16, tag="xt")
                                nc.sync.dma_start(
                                    t_[:],
                                    xt_d.ap()[d * 128:(d + 1) * 128,
                                              c4 * 512:(c4 + 1) * 512])
                                xp.append(t_)
                            qstate["xp"], qstate["c4"] = xp, c4
                        ps = psq.tile([128, 512], f32, tag="psqkv")
                        for d in range(KD):
                            nc.tensor.matmul(
                                ps[:],
                                wqt[:, d * HD + t * 128:
                                    d * HD + (t + 1) * 128],
                                qstate["xp"][d][:],
                                start=(d == 0), stop=(d == KD - 1))
                        nc.vector.tensor_scalar_add(
                            qT[:, t * S + c4 * 512:
                               t * S + (c4 + 1) * 512],
                            ps[:], bq_t[:, t:t + 1])

                    def q_chunk(c4):
                        for t in range(NT):
                            qproj_t(c4, t)

                    def v_chunk(m):
                        # V projection for kv chunk m: lhsT = xk [d,128 kv],
                        # rhs = wvt -> psum [128 kv, 512]; evict into the
                        # per-head v columns 64:128 of vA (ones block kept).
                        # xkT loads are batched per piece and sliced per
                        # chunk (fewer, larger DMAs).
                        off = 0
                        for pi, (poff, w) in enumerate(kv_pieces):
                            if poff <= m * 128 < poff + w:
                                off, piece = poff, pi
                                break
                        mi = m - off // 128
                        ps = psq.tile([128, 512], f32, tag="psqkv")
                        for d in range(KD):
                            nc.tensor.matmul(
                                ps[:],
                                xk_pieces[piece][d][:, mi * 128:
                                                    (mi + 1) * 128],
                                wvt[:, d * HD:(d + 1) * HD],
                                start=(d == 0),
                                stop=(not has_bv and d == KD - 1))
                        if has_bv:
                            nc.tensor.matmul(
                                ps[:], ones_r[:], bv_r[:],
                                start=False, stop=True)
                        nc.vector.tensor_copy(
                            vA[:, m * HPC * VW:(m + 1) * HPC * VW]
                            .rearrange("p (h e) -> p h e", h=HPC)
                            [:, :, DH:VW],
                            ps[:].rearrange("p (h e) -> p h e", h=HPC))

                    def attn(g, c, with_v):
                        ctxA = psctx.tile([128, 512], f32, tag="ctx")
                        ctxB = psctx.tile([128, 512], f32, tag="ctx")
                        for m in range(nmk_attn):
                            if with_v:
                                v_chunk(m)
                            sc = pssc.tile([128, 1024], f32, tag="sc")
                            # scoresT for the pair, row-packed on the PE
                            nc.tensor.matmul(
                                sc[:, 0:512],
                                kT[0:64, g * nkv + m * 128:
                                   g * nkv + (m + 1) * 128],
                                qT[0:64, g * S + c * 512:
                                   g * S + (c + 1) * 512],
                                start=True, stop=True)
                            nc.tensor.matmul(
                                sc[:, 512:1024],
                                kT[64:128, g * nkv + m * 128:
                                   g * nkv + (m + 1) * 128],
                                qT[64:128, g * S + c * 512:
                                   g * S + (c + 1) * 512],
                                start=True, stop=True)
                            probs = ppool.tile([128, 1024], f32r,
                                               tag="probs")
                            nc.scalar.activation(
                                probs[:], sc[:], EXP,
                                bias=adder[:, m:m + 1], scale=0.125)
                            hA, hB = 2 * g, 2 * g + 1
                            nc.tensor.matmul(
                                ctxA[:],
                                vA[:, (m * HPC + hA) * VW:
                                   (m * HPC + hA + 1) * VW],
                                probs[:, 0:512],
                                start=(m == 0), stop=(m == nmk_attn - 1))
                            nc.tensor.matmul(
                                ctxB[:],
                                vA[:, (m * HPC + hB) * VW:
                                   (m * HPC + hB + 1) * VW],
                                probs[:, 512:1024],
                                start=(m == 0), stop=(m == nmk_attn - 1))

                        # rows 0:63 of each ctx tile hold the denominator
                        # already broadcast to 64 partitions (ones block of
                        # vA), rows 64:127 the raw context. Evict both heads
                        # into one SBUF tile (frees the PSUM banks), then a
                        # single reciprocal / crossing copy / multiply for
                        # the pair, and one DMA per head.
                        u = rcpool.tile([128, 1024], f32, tag="u")
                        nc.vector.tensor_copy(u[:, 0:512], ctxA[:])
                        nc.vector.tensor_copy(u[:, 512:1024], ctxB[:])
                        bcs = bcpool.tile([DH, 1024], f32, tag="bcs")
                        nc.vector.reciprocal_approx_fast(
                            out=bcs[:], in_=u[0:DH, :])
                        uc = bcpool.tile([DH, 1024], f32, tag="uc")
                        nc.vector.tensor_copy(uc[:], u[DH:2 * DH, :])
                        o = opool.tile([DH, 1024], f32, tag="o")
                        nc.vector.tensor_mul(o[:], uc[:], bcs[:])
                        nc.sync.dma_start(
                            out_d.ap()[2 * g * DH:(2 * g + 2) * DH,
                                       c * 512:(c + 1) * 512]
                            .rearrange("(i p) e -> p i e", i=2),
                            o[:].rearrange("p (i e) -> p i e", i=2))

                    # ---------------- emission order ----------------
                    q_chunk(0)
                    dma_w(wvt, wv_d)
                    for c in range(NC4):
                        for g in range(NT):
                            if c + 1 < NC4:
                                qproj_t(c + 1, g)
                            attn(g, c, with_v=(c == 0 and g == 0))
                    if taps:
                        nc.sync.dma_start(dva_d.ap(), vA[:].bitcast(f32))

    nc.compile()
    return nc


def get_nc(nkv, nmk_attn, has_bv=True):
    key = (nkv, nmk_attn, has_bv)
    if key not in _CACHED:
        _CACHED[key] = _build_nc(nkv, nmk_attn, has_bv=has_bv)
    return _CACHED[key]


def make_in_maps(nkv, x, mask, wq, bq, wk, bk, wv, bv):
    import ml_dtypes

    bf16 = ml_dtypes.bfloat16
    x = np.ascontiguousarray(np.asarray(x, dtype=np.float32))
    mask = np.ascontiguousarray(np.asarray(mask, dtype=np.int32))
    wq = np.asarray(wq, dtype=np.float32)
    wk = np.asarray(wk, dtype=np.float32)
    wv = np.asarray(wv, dtype=np.float32)
    bq = np.asarray(bq, dtype=np.float32)
    bk = np.asarray(bk, dtype=np.float32)
    bv = np.asarray(bv, dtype=np.float32)
    # per-batch kv compaction indices (unmasked first, masked as padding)
    idx = []
    for b in range(B):
        on = np.flatnonzero(mask[b] != 0)
        off = np.flatnonzero(mask[b] == 0)
        ib = np.concatenate([on, off])[:nkv]
        idx.append(ib)
    in_maps = []
    for c in range(NCORES):
        b, g = c // 2, c % 2
        cols = slice(g * HD, (g + 1) * HD)
        xtb = np.ascontiguousarray(x[b].T.astype(bf16))
        in_maps.append({
            "xt": xtb,
            "xkt": np.ascontiguousarray(xtb[:, idx[b]]),
            "wq": np.ascontiguousarray(wq[:, cols].astype(bf16)),
            "wk": np.ascontiguousarray(wk[:, cols].astype(bf16)),
            "wv": np.ascontiguousarray(wv[:, cols].astype(bf16)),
            "bq": np.ascontiguousarray(bq[cols]),
            "bk": np.ascontiguousarray(bk[cols]),
            "bv": np.ascontiguousarray(bv[cols]),
            "maskc": np.ascontiguousarray(mask[b][idx[b]]),
        })
    return in_maps


def assemble_out(results):
    out = np.empty((B, S, H * DH), dtype=np.float32)
    for c in range(NCORES):
        b, g = c // 2, c % 2
        out[b, :, g * HD:(g + 1) * HD] = results[c]["out"].T
    return out


def pick_nkv(mask):
    mask = np.asarray(mask)
    nb_max = int((mask != 0).sum(axis=1).max())
    nmk_attn = max(2, -(-nb_max // 128))
    nkv = min(nmk_attn * 128, S)
    return nkv, nmk_attn


def run(trace=False, **inputs):
    from concourse.bass_utils import run_bass_kernel_spmd

    nkv, nmk_attn = pick_nkv(inputs["mask"])
    has_bv = bool(np.any(np.asarray(inputs["bv"])))
    nc = get_nc(nkv, nmk_attn, has_bv)
    in_maps = make_in_maps(nkv, **inputs)
    if not has_bv:
        for m in in_maps:
            m.pop("bv", None)
    res = run_bass_kernel_spmd(nc, in_maps, core_ids=list(range(NCORES)),
                               trace=trace)
    return assemble_out(res.results), res


def kernel(**inputs):
    out, _ = run(trace=False, **inputs)
    return out
